# revision 24
# baseline (speedup 1.0000x reference)
"""Multi-head attention (16 heads, E=1024, seq=2048, batch=4) on 8 NeuronCores.

Sharding: core = 2*b + g  (b = batch 0..3, g = head-group 0..1, 8 heads each).
Each core computes its batch's QKV for its 8 heads, attention, and a partial
output projection (rows of W_out for its heads); host sums the two partials
per batch and adds b_out.

On-chip layout avoids all transposes:
  - host supplies x^T [1024, 2048] per core
  - q^T,k^T computed as (W^T x^T)  -> [qk_col, seq]   (lhsT = W chunk)
  - v computed naturally as x @ W_v -> [seq, v_col]   (lhsT = x^T chunk)
  - scores^T[sk, sq] = (k^T chunk)^T.T @ q^T  (lhsT = k^T slice, rhs = q^T);
    head pairs share one PSUM tile ([A sq512 | B sq512]) with the two
    64-contraction matmuls row-packed via tile_position, so one Exp
    activation covers both heads
  - softmax denominator via an appended ones-column in the PV lhsT
  - PV: out^T[d(+1), sq] = [v | 1]^T @ attn^T, accumulated over sk chunks
  - normalize: denominator row broadcast across partitions with a K=1
    matmul, then reciprocal+multiply on DVE (deferred into the next
    sq-block's chunk stream so the PE never waits on the chain)
  - proj: y[sq, :] from lhsT = out^T tiles, rhs = W_out rows for this group

Scheduling: phase B (attention) is ACT-bound on the Exp stream, so the
QKV projections for head pair p+1 and the final projection are dribbled
into pair p's / pair 3's chunk stream one matmul at a time to fill the
PE slack. All matmuls run in float32r (TF32-like, full rate at N=512)
with fp32 PSUM accumulate.
"""

import sys

sys.path.insert(0, "/opt/trn_rl_repo")

import ml_dtypes
import numpy as np

BF16_NP = ml_dtypes.bfloat16

import concourse.bacc as bacc
import concourse.mybir as mybir
import concourse.tile as tile
from concourse import bass_utils

P = 128
SEQ = 2048
EMB = 1024
N_HEADS_CORE = 8
D_HEAD = 64
QK_COLS = 1024          # q(512) + k(512) for this core's heads
V_COLS = 512
VA = D_HEAD + 1         # v columns per head incl. ones column
N_CORES = 8
NORM = 0.125            # 1/sqrt(64), folded into W_q/b_q on host

F32 = mybir.dt.float32
F32R = mybir.dt.float32r
BF16 = mybir.dt.bfloat16
AF = mybir.ActivationFunctionType

_CACHED = None


def _build():
    nc = bacc.Bacc("TRN2", target_bir_lowering=False, debug=False,
                   enable_asserts=True, num_devices=N_CORES)

    xT = nc.dram_tensor("xT", [EMB, SEQ], BF16, kind="ExternalInput").ap()
    wqk = nc.dram_tensor("wqk", [EMB, QK_COLS], BF16, kind="ExternalInput").ap()
    wv = nc.dram_tensor("wv", [EMB, V_COLS], BF16, kind="ExternalInput").ap()
    wo = nc.dram_tensor("wo", [V_COLS, EMB], F32R, kind="ExternalInput").ap()
    bqk = nc.dram_tensor("bqk", [P, QK_COLS // P], F32, kind="ExternalInput").ap()
    bv = nc.dram_tensor("bv", [1, V_COLS], F32, kind="ExternalInput").ap()
    out = nc.dram_tensor("out", [SEQ, EMB], BF16, kind="ExternalOutput").ap()
    # DRAM bounce buffer for the softmax denominator rows: SBUF sources
    # cannot do stride-0 partition broadcast, DRAM sources can
    dnb = nc.dram_tensor("dnb", [2, 512], F32, kind="Internal").ap()

    KC = EMB // P          # 8 contraction chunks
    NSC = SEQ // P         # 16 seq chunks of 128
    NJB = SEQ // 512       # 4 sq blocks of 512

    with tile.TileContext(nc) as tc:
      with tc.tile_pool(name="persist", bufs=1) as persist, \
           tc.tile_pool(name="qkT", bufs=2) as qkT_pool, \
           tc.tile_pool(name="oTp", bufs=1) as oT_pool, \
           tc.tile_pool(name="attn", bufs=2) as attn_pool, \
           tc.tile_pool(name="nrm", bufs=2) as nrm_pool, \
           tc.tile_pool(name="ps_s", bufs=2, space="PSUM") as ps_s_pool, \
           tc.tile_pool(name="ps_o0", bufs=1, space="PSUM") as ps_o0_pool, \
           tc.tile_pool(name="ps_o1", bufs=1, space="PSUM") as ps_o1_pool:
        ps_o_pools = [ps_o0_pool, ps_o1_pool]
        vsb = [persist.tile([P, N_HEADS_CORE * VA], F32R, tag=f"v{s}", name=f"v{s}")
               for s in range(NSC)]
        bqk_sb = persist.tile([P, QK_COLS // P], F32, tag="bqk")
        bv_sb = persist.tile([P, V_COLS], F32, tag="bv")
        nc.sync.dma_start(bqk_sb[:], bqk)
        nc.sync.dma_start(bv_sb[:], bv[0:1, :].broadcast_to([P, V_COLS]))

        qT = {}
        kT = {}
        outT = [oT_pool.tile([P, SEQ], F32R, tag=f"oT{t}", name=f"oT{t}")
                for t in range(4)]

        from collections import deque
        pending = deque()

        def emit_B_pair(t, fillers, scratch_pool, after_jb=None, dynamic=False,
                        flush=False):
            """Head pair (2t, 2t+1): rows 0-63 / 64-127 of qT[t]/kT[t].
            Per chunk one ps_s [128,1024] = [A sq512 | B sq512]; scores
            row-packed, one exp for both heads, PV splits to per-head
            accumulators. `fillers` are thunks sprinkled into the chunk
            stream to fill PE slack under the ACT-bound exp pipeline."""
            kTh = kT[t]
            qTh = qT[t]
            it = 0
            fi = 0
            nfill = len(fillers)
            for j in range(NJB):
                sq0 = j * 512
                ps_os = [ps_o_pools[hh].tile([VA, 512], F32, tag=f"ps_o{hh}",
                                             name=f"ps_o{t}_{j}_{hh}")
                         for hh in range(2)]

                def scores(c):
                    ps_s = ps_s_pool.tile([P, 2 * 512], F32, tag="ps_s",
                                          name=f"ps_s{t}_{j}_{c}")
                    for hh in range(2):
                        pr = hh * D_HEAD
                        nc.tensor.matmul(
                            ps_s[:, hh * 512:(hh + 1) * 512],
                            kTh[pr:pr + D_HEAD, c * P:(c + 1) * P],
                            qTh[pr:pr + D_HEAD, sq0:sq0 + 512],
                            start=True, stop=True, tile_position=(pr, 0))
                    return ps_s

                ps_s = scores(0)
                for c in range(NSC):
                    at = attn_pool.tile([P, 2 * 512], F32R, tag="attnT",
                                        name=f"at{t}_{j}_{c}")
                    nc.scalar.activation(at[:], ps_s[:], AF.Exp)
                    if c + 1 < NSC:
                        ps_s = scores(c + 1)
                    va3 = vsb[c][:].rearrange("p (h c) -> p h c", c=VA)
                    for hh in range(2):
                        nc.tensor.matmul(
                            ps_os[hh][:],
                            va3[:, 2 * t + hh, :],
                            at[:, hh * 512:(hh + 1) * 512],
                            start=(c == 0), stop=(c == NSC - 1))
                    it += 1
                    if c >= 2:
                        for _ in range(2):
                            if pending:
                                pending.popleft()()
                    if dynamic:
                        budget = 3
                        while fi < len(fillers) and budget > 0:
                            fillers[fi]()
                            fi += 1
                            budget -= 1
                    else:
                        while nfill and fi < (nfill * it) // 64 and fi < nfill:
                            fillers[fi]()
                            fi += 1

                # stage 1 (DVE): evacuate ps_o; broadcast the denominator
                # row across 64 partitions with a stride-0 SBUF->SBUF DMA
                # (keeps the normalize chain off PSUM and off the PE)
                outUs = []
                rcasts = []
                for hh in range(2):
                    outU = nrm_pool.tile([VA, 512], F32, tag=f"outU{hh}",
                                         name=f"outU{t}_{j}_{hh}", bufs=1)
                    nc.vector.tensor_copy(outU[:], ps_os[hh][:])
                    rbb = nrm_pool.tile([D_HEAD, 512], F32, tag=f"rbb{hh}",
                                        name=f"rbb{t}_{j}_{hh}", bufs=1)
                    # gpsimd DMA queue: the sync queue carries the output
                    # tiles in pair 3 and would delay this bounce by ~4us
                    nc.gpsimd.dma_start(dnb[hh:hh + 1, :], outU[D_HEAD:VA, :])
                    nc.gpsimd.dma_start(
                        rbb[:], dnb[hh:hh + 1, :].broadcast_to([D_HEAD, 512]))
                    outUs.append(outU)
                    rcasts.append(rbb)

                # stage 2, deferred into the next j-block's chunk stream as
                # small steps: reciprocal+multiply in [64,128] column slices
                # so no single DVE op can block the FIFO ahead of the PSUM
                # evacuations that gate the PE.
                def make_fin_steps(jj, sq00, oUs, rcs):
                    def recmul(hh, p):
                        def go():
                            cs = slice(p * 128, (p + 1) * 128)
                            rb = nrm_pool.tile([D_HEAD, 128], F32,
                                               tag=f"rb{hh}_{p}",
                                               name=f"rb{t}_{jj}_{hh}_{p}",
                                               bufs=1)
                            nc.vector.reciprocal(rb[:], rcs[hh][:, cs])
                            nc.vector.tensor_mul(
                                outT[t][hh * D_HEAD:(hh + 1) * D_HEAD,
                                        sq00 + p * 128:sq00 + (p + 1) * 128],
                                oUs[hh][0:D_HEAD, cs], rb[:])
                        return go

                    steps = []
                    for p in range(4):
                        for hh in range(2):
                            steps.append(recmul(hh, p))
                    if after_jb is not None:
                        steps.append(lambda: fillers.extend(after_jb(jj)))
                    return steps

                pending.extend(make_fin_steps(j, sq0, outUs, rcasts))
            # drain: alternate steps and fillers so the tail's proj matmuls
            # (extended by the last step) overlap the DVE normalize chain
            while (flush and pending) or fi < len(fillers):
                if flush and pending:
                    pending.popleft()()
                for _ in range(3):
                    if fi < len(fillers):
                        fillers[fi]()
                        fi += 1

        # ---- phase A scaffolding (xT, wv, rotating weight slices) ----
        with tc.tile_pool(name="xTp", bufs=1) as xTp, \
             tc.tile_pool(name="wvp", bufs=1) as wvp, \
             tc.tile_pool(name="wrot", bufs=2) as wrot, \
             tc.tile_pool(name="psA", bufs=2, space="PSUM") as psA:
            xT_sb = [xTp.tile([P, SEQ], BF16, tag=f"xT{k}", name=f"xTs{k}")
                     for k in range(KC)]
            wv_sb = [wvp.tile([P, V_COLS], BF16, tag=f"wv{k}", name=f"wvs{k}")
                     for k in range(KC)]
            # sq-block-sliced so the first qk column groups can start after
            # 2MB instead of the full 8MB of x^T
            for jsl in range(NJB):
                for k in range(KC):
                    nc.sync.dma_start(
                        xT_sb[k][:, jsl * 512:(jsl + 1) * 512],
                        xT[k * P:(k + 1) * P, jsl * 512:(jsl + 1) * 512])
            for k in range(KC):
                nc.sync.dma_start(wv_sb[k][:], wv[k * P:(k + 1) * P, :])

            def emit_wr_dma(t):
                wt = []
                for k in range(KC):
                    w = wrot.tile([P, P], BF16, tag=f"wr{k}", name=f"wr{t}_{k}")
                    nc.gpsimd.dma_start(
                        w[:], wqk[k * P:(k + 1) * P, t * P:(t + 1) * P])
                    wt.append(w)
                return wt

            def emit_qk_col(t, wt, fine=False):
                """One column tile of q^T (t<4) or k^T (t>=4): 4 psum groups.
                fine=True returns one thunk per matmul (32 thunks) so a
                group can be dribbled into B's per-chunk PE slack."""
                if t < 4:
                    dst = qT[t] = qkT_pool.tile([P, SEQ], F32R, tag="qTa",
                                                name=f"qT{t}")
                else:
                    dst = kT[t - 4] = qkT_pool.tile([P, SEQ], F32R, tag="kTa",
                                                    name=f"kT{t-4}")

                state = {}

                def one_mm(j, k):
                    def go():
                        if k == 0:
                            state[j] = psA.tile([P, 512], F32, tag="psA_t",
                                                name=f"psqk{t}_{j}")
                        ps = state[j]
                        nc.tensor.matmul(
                            ps[:], wt[k][:],
                            xT_sb[k][:, j * 512:(j + 1) * 512],
                            start=(k == 0), stop=(k == KC - 1))
                        if k == KC - 1:
                            nc.vector.tensor_scalar_add(
                                dst[:, j * 512:(j + 1) * 512], ps[:],
                                bqk_sb[:, t:t + 1])
                    return go

                def one_group(j):
                    def go():
                        for k in range(KC):
                            one_mm(j, k)()
                    return go

                if fine:
                    return [one_mm(j, k) for j in range(NJB) for k in range(KC)]
                return [one_group(j) for j in range(NJB)]

            def emit_v_group(s):
                ps = psA.tile([P, V_COLS], F32, tag="psA_t", name=f"psv{s}")
                for k in range(KC):
                    nc.tensor.matmul(
                        ps[:],
                        xT_sb[k][:, s * P:(s + 1) * P],
                        wv_sb[k][:],
                        start=(k == 0), stop=(k == KC - 1))
                v3 = vsb[s][:].rearrange("p (h c) -> p h c", c=VA)
                ps3 = ps[:].rearrange("p (h c) -> p h c", c=D_HEAD)
                bv3 = bv_sb[:].rearrange("p (h c) -> p h c", c=D_HEAD)
                nc.vector.tensor_add(v3[:, :, 0:D_HEAD], ps3, bv3)
                nc.vector.tensor_scalar(
                    v3[:, :, D_HEAD], bv_sb[:, 0:N_HEADS_CORE], 0.0, 1.0,
                    mybir.AluOpType.mult, mybir.AluOpType.add)

            # head: q^T/k^T for pair 0 and v, emitted j-major and
            # interleaved per xT slice so the PE starts after ~1MB of
            # x^T DMA instead of all 4MB
            wt01 = {}
            wt0 = emit_wr_dma(0)
            wt4 = emit_wr_dma(4)
            wt01[1] = emit_wr_dma(1)   # prefetch pair0's first filler weights
            q_groups = emit_qk_col(0, wt0)
            k_groups = emit_qk_col(4, wt4)
            for j in range(NJB):
                q_groups[j]()
                k_groups[j]()
                for s in range(4 * j, 4 * j + 4):
                    emit_v_group(s)

            # B pairs 0-2, with pair p+1's q^T/k^T production dribbled
            # into the chunk stream one matmul at a time; weight slices
            # prefetched one pair ahead
            for pair in range(3):
                t_lo, t_hi = pair + 1, pair + 5
                wt_lo = wt01.pop(t_lo, None) or emit_wr_dma(t_lo)
                fillers = list(emit_qk_col(t_lo, wt_lo, fine=True))
                wt_hi = emit_wr_dma(t_hi)
                if pair < 2:
                    wt01[pair + 2] = emit_wr_dma(pair + 2)
                fillers.extend(emit_qk_col(t_hi, wt_hi, fine=True))
                emit_B_pair(pair, fillers, psA)

        # ---- pair 3 + projection (xT/wv freed; wo loads into that space)
        with tc.tile_pool(name="wop", bufs=1) as wop, \
             tc.tile_pool(name="osb", bufs=2) as osb_pool, \
             tc.tile_pool(name="psC", bufs=2, space="PSUM") as psC:
            wo_sb = [wop.tile([P, EMB], F32R, tag=f"wo{t}", name=f"wo{t}")
                     for t in range(4)]
            for t in range(4):
                nc.sync.dma_start(wo_sb[t][:], wo[t * P:(t + 1) * P, :])

            cstate = {}

            def one_c_mm(s, y, t):
                def go():
                    if t == 0:
                        cstate[(s, y)] = psC.tile([P, 512], F32, tag="psC_t",
                                                  name=f"psc{s}_{y}")
                    ps = cstate[(s, y)]
                    nc.tensor.matmul(
                        ps[:],
                        outT[t][:, s * P:(s + 1) * P],
                        wo_sb[t][:, y * 512:(y + 1) * 512],
                        start=(t == 0), stop=(t == 3))
                    if t == 3:
                        ot = osb_pool.tile([P, 512], BF16, tag="osb",
                                           name=f"osb{s}_{y}")
                        nc.vector.tensor_copy(ot[:], ps[:])
                        nc.sync.dma_start(
                            out[s * P:(s + 1) * P, y * 512:(y + 1) * 512],
                            ot[:])
                return go

            def emit_C_jb(jb):
                return [one_c_mm(s, y, t)
                        for s in range(4 * jb, 4 * jb + 4)
                        for y in range(EMB // 512)
                        for t in range(4)]

            emit_B_pair(3, [], psC, after_jb=emit_C_jb, dynamic=True, flush=True)

    nc.compile()
    return nc


def get_nc():
    global _CACHED
    if _CACHED is None:
        _CACHED = _build()
    return _CACHED


def make_in_maps(x, W_qkv, b_qkv, W_out, b_out):
    x = np.asarray(x, dtype=np.float32)
    W_qkv = np.asarray(W_qkv, dtype=np.float32)
    b_qkv = np.asarray(b_qkv, dtype=np.float32)
    W_out = np.asarray(W_out, dtype=np.float32)
    b_out = np.asarray(b_out, dtype=np.float32)

    in_maps = []
    for core in range(N_CORES):
        b, g = divmod(core, 2)
        c0 = g * 512
        wq = W_qkv[:, c0:c0 + 512] * NORM
        wk = W_qkv[:, EMB + c0:EMB + c0 + 512]
        wv_ = W_qkv[:, 2 * EMB + c0:2 * EMB + c0 + 512]
        bq = b_qkv[c0:c0 + 512] * NORM
        bk = b_qkv[EMB + c0:EMB + c0 + 512]
        bv_ = b_qkv[2 * EMB + c0:2 * EMB + c0 + 512]
        in_maps.append({
            "xT": np.ascontiguousarray(x[b].T).astype(BF16_NP),
            "wqk": np.ascontiguousarray(
                np.concatenate([wq, wk], axis=1)).astype(BF16_NP),
            "wv": np.ascontiguousarray(wv_).astype(BF16_NP),
            "wo": np.ascontiguousarray(W_out[c0:c0 + 512, :]),
            "bqk": np.ascontiguousarray(
                np.concatenate([bq, bk]).reshape(QK_COLS // P, P).T),
            "bv": bv_.reshape(1, V_COLS),
        })
    return in_maps


def kernel(x, W_qkv, b_qkv, W_out, b_out):
    nc = get_nc()
    b_out = np.asarray(b_out, dtype=np.float32)
    in_maps = make_in_maps(x, W_qkv, b_qkv, W_out, b_out)
    res = bass_utils.run_bass_kernel_spmd(nc, in_maps, core_ids=list(range(N_CORES)))
    outp = np.empty((4, SEQ, EMB), dtype=np.float32)
    for b in range(4):
        outp[b] = (res.results[2 * b]["out"].astype(np.float32)
                   + res.results[2 * b + 1]["out"].astype(np.float32) + b_out)
    return outp



# revision 30
# speedup vs baseline: 1.0000x; 1.0000x over previous
"""Multi-head attention (16 heads, E=1024, seq=2048, batch=4) on 8 NeuronCores.

Sharding: core = 2*b + g  (b = batch 0..3, g = head-group 0..1, 8 heads each).
Each core computes its batch's QKV for its 8 heads, attention, and a partial
output projection (rows of W_out for its heads); host sums the two partials
per batch and adds b_out.

On-chip layout avoids all transposes:
  - host supplies x^T [1024, 2048] per core
  - q^T,k^T computed as (W^T x^T)  -> [qk_col, seq]   (lhsT = W chunk)
  - v computed naturally as x @ W_v -> [seq, v_col]   (lhsT = x^T chunk)
  - scores^T[sk, sq] = (k^T chunk)^T.T @ q^T  (lhsT = k^T slice, rhs = q^T);
    head pairs share one PSUM tile ([A sq512 | B sq512]) with the two
    64-contraction matmuls row-packed via tile_position, so one Exp
    activation covers both heads
  - softmax denominator via an appended ones-column in the PV lhsT
  - PV: out^T[d(+1), sq] = [v | 1]^T @ attn^T, accumulated over sk chunks
  - normalize: denominator row broadcast across partitions with a K=1
    matmul, then reciprocal+multiply on DVE (deferred into the next
    sq-block's chunk stream so the PE never waits on the chain)
  - proj: y[sq, :] from lhsT = out^T tiles, rhs = W_out rows for this group

Scheduling: phase B (attention) is ACT-bound on the Exp stream, so the
QKV projections for head pair p+1 and the final projection are dribbled
into pair p's / pair 3's chunk stream one matmul at a time to fill the
PE slack. All matmuls run in float32r (TF32-like, full rate at N=512)
with fp32 PSUM accumulate.
"""

import sys

sys.path.insert(0, "/opt/trn_rl_repo")

import ml_dtypes
import numpy as np

BF16_NP = ml_dtypes.bfloat16

import concourse.bacc as bacc
import concourse.mybir as mybir
import concourse.tile as tile
from concourse import bass_utils

P = 128
SEQ = 2048
EMB = 1024
N_HEADS_CORE = 8
D_HEAD = 64
QK_COLS = 1024          # q(512) + k(512) for this core's heads
V_COLS = 512
VA = D_HEAD + 1         # v columns per head incl. ones column
N_CORES = 8
NORM = 0.125            # 1/sqrt(64), folded into W_q/b_q on host

F32 = mybir.dt.float32
F32R = mybir.dt.float32r
BF16 = mybir.dt.bfloat16
AF = mybir.ActivationFunctionType

_CACHED = None


def _build():
    nc = bacc.Bacc("TRN2", target_bir_lowering=False, debug=False,
                   enable_asserts=True, num_devices=N_CORES)

    xT = nc.dram_tensor("xT", [EMB, SEQ], BF16, kind="ExternalInput").ap()
    wqk = nc.dram_tensor("wqk", [EMB, QK_COLS], BF16, kind="ExternalInput").ap()
    wv = nc.dram_tensor("wv", [EMB, V_COLS], BF16, kind="ExternalInput").ap()
    wo = nc.dram_tensor("wo", [V_COLS, EMB], F32R, kind="ExternalInput").ap()
    bqk = nc.dram_tensor("bqk", [P, QK_COLS // P], F32, kind="ExternalInput").ap()
    bv = nc.dram_tensor("bv", [1, V_COLS], F32, kind="ExternalInput").ap()
    out = nc.dram_tensor("out", [SEQ, EMB], BF16, kind="ExternalOutput").ap()
    # DRAM bounce buffer for the softmax denominator rows: SBUF sources
    # cannot do stride-0 partition broadcast, DRAM sources can
    dnb = nc.dram_tensor("dnb", [2, 512], F32, kind="Internal").ap()

    KC = EMB // P          # 8 contraction chunks
    NSC = SEQ // P         # 16 seq chunks of 128
    NJB = SEQ // 512       # 4 sq blocks of 512

    with tile.TileContext(nc) as tc:
      with tc.tile_pool(name="persist", bufs=1) as persist, \
           tc.tile_pool(name="qkT", bufs=2) as qkT_pool, \
           tc.tile_pool(name="oTp", bufs=1) as oT_pool, \
           tc.tile_pool(name="attn", bufs=2) as attn_pool, \
           tc.tile_pool(name="nrm", bufs=2) as nrm_pool, \
           tc.tile_pool(name="ps_s", bufs=2, space="PSUM") as ps_s_pool, \
           tc.tile_pool(name="ps_o0", bufs=1, space="PSUM") as ps_o0_pool, \
           tc.tile_pool(name="ps_o1", bufs=1, space="PSUM") as ps_o1_pool:
        ps_o_pools = [ps_o0_pool, ps_o1_pool]
        vsb = [persist.tile([P, N_HEADS_CORE * VA], F32R, tag=f"v{s}", name=f"v{s}")
               for s in range(NSC)]
        bqk_sb = persist.tile([P, QK_COLS // P], F32, tag="bqk")
        bv_sb = persist.tile([P, V_COLS], F32, tag="bv")
        nc.sync.dma_start(bqk_sb[:], bqk)
        nc.sync.dma_start(bv_sb[:], bv[0:1, :].broadcast_to([P, V_COLS]))

        qT = {}
        kT = {}
        outT = [oT_pool.tile([P, SEQ], F32R, tag=f"oT{t}", name=f"oT{t}")
                for t in range(4)]

        from collections import deque
        pending = deque()

        def emit_B_pair(t, fillers, scratch_pool, after_jb=None, dynamic=False,
                        flush=False):
            """Head pair (2t, 2t+1): rows 0-63 / 64-127 of qT[t]/kT[t].
            Per chunk one ps_s [128,1024] = [A sq512 | B sq512]; scores
            row-packed, one exp for both heads, PV splits to per-head
            accumulators. `fillers` are thunks sprinkled into the chunk
            stream to fill PE slack under the ACT-bound exp pipeline."""
            kTh = kT[t]
            qTh = qT[t]
            it = 0
            fi = 0
            nfill = len(fillers)
            for j in range(NJB):
                sq0 = j * 512
                ps_os = [ps_o_pools[hh].tile([VA, 512], F32, tag=f"ps_o{hh}",
                                             name=f"ps_o{t}_{j}_{hh}")
                         for hh in range(2)]

                def scores(c):
                    ps_s = ps_s_pool.tile([P, 2 * 512], F32, tag="ps_s",
                                          name=f"ps_s{t}_{j}_{c}")
                    for hh in range(2):
                        pr = hh * D_HEAD
                        nc.tensor.matmul(
                            ps_s[:, hh * 512:(hh + 1) * 512],
                            kTh[pr:pr + D_HEAD, c * P:(c + 1) * P],
                            qTh[pr:pr + D_HEAD, sq0:sq0 + 512],
                            start=True, stop=True, tile_position=(pr, 0))
                    return ps_s

                ps_s = scores(0)
                for c in range(NSC):
                    at = attn_pool.tile([P, 2 * 512], F32R, tag="attnT",
                                        name=f"at{t}_{j}_{c}")
                    nc.scalar.activation(at[:], ps_s[:], AF.Exp)
                    if c + 1 < NSC:
                        ps_s = scores(c + 1)
                    va3 = vsb[c][:].rearrange("p (h c) -> p h c", c=VA)
                    for hh in range(2):
                        nc.tensor.matmul(
                            ps_os[hh][:],
                            va3[:, 2 * t + hh, :],
                            at[:, hh * 512:(hh + 1) * 512],
                            start=(c == 0), stop=(c == NSC - 1))
                    it += 1
                    if c >= 2:
                        for _ in range(4):
                            if pending:
                                pending.popleft()()
                    if dynamic:
                        budget = 3
                        while fi < len(fillers) and budget > 0:
                            fillers[fi]()
                            fi += 1
                            budget -= 1
                    else:
                        while nfill and fi < (nfill * it) // 64 and fi < nfill:
                            fillers[fi]()
                            fi += 1

                # stage 1 (DVE): evacuate ps_o; broadcast the denominator
                # row across 64 partitions with a stride-0 SBUF->SBUF DMA
                # (keeps the normalize chain off PSUM and off the PE)
                outUs = []
                rcasts = []
                for hh in range(2):
                    outU = nrm_pool.tile([VA, 512], F32, tag=f"outU{hh}",
                                         name=f"outU{t}_{j}_{hh}", bufs=1)
                    nc.vector.tensor_copy(outU[:], ps_os[hh][:])
                    rbb = nrm_pool.tile([D_HEAD, 512], F32, tag=f"rbb{hh}",
                                        name=f"rbb{t}_{j}_{hh}", bufs=1)
                    # gpsimd DMA queue: the sync queue carries the output
                    # tiles in pair 3 and would delay this bounce by ~4us
                    nc.gpsimd.dma_start(dnb[hh:hh + 1, :], outU[D_HEAD:VA, :])
                    nc.gpsimd.dma_start(
                        rbb[:], dnb[hh:hh + 1, :].broadcast_to([D_HEAD, 512]))
                    outUs.append(outU)
                    rcasts.append(rbb)

                # stage 2, deferred into the next j-block's chunk stream as
                # small steps: reciprocal+multiply in [64,128] column slices
                # so no single DVE op can block the FIFO ahead of the PSUM
                # evacuations that gate the PE.
                def make_fin_steps(jj, sq00, oUs, rcs):
                    def recmul(hh, p):
                        def go():
                            cs = slice(p * 128, (p + 1) * 128)
                            rb = nrm_pool.tile([D_HEAD, 128], F32,
                                               tag=f"rb{hh}_{p}",
                                               name=f"rb{t}_{jj}_{hh}_{p}",
                                               bufs=1)
                            nc.vector.reciprocal(rb[:], rcs[hh][:, cs])
                            nc.vector.tensor_mul(
                                outT[t][hh * D_HEAD:(hh + 1) * D_HEAD,
                                        sq00 + p * 128:sq00 + (p + 1) * 128],
                                oUs[hh][0:D_HEAD, cs], rb[:])
                        return go

                    steps = []
                    for p in range(4):
                        for hh in range(2):
                            steps.append(recmul(hh, p))
                        if after_jb is not None:
                            # proj thunks for this column slice right after
                            # the muls that produce their outT inputs
                            steps.extend(after_jb(4 * jj + p))
                    return steps

                pending.extend(make_fin_steps(j, sq0, outUs, rcasts))
            # drain: alternate steps and fillers so the tail's proj matmuls
            # (extended by the last step) overlap the DVE normalize chain
            while (flush and pending) or fi < len(fillers):
                if flush and pending:
                    pending.popleft()()
                for _ in range(3):
                    if fi < len(fillers):
                        fillers[fi]()
                        fi += 1

        # ---- phase A scaffolding (xT, wv, rotating weight slices) ----
        with tc.tile_pool(name="xTp", bufs=1) as xTp, \
             tc.tile_pool(name="wvp", bufs=1) as wvp, \
             tc.tile_pool(name="wrot", bufs=2) as wrot, \
             tc.tile_pool(name="psA", bufs=2, space="PSUM") as psA:
            xT_sb = [xTp.tile([P, SEQ], BF16, tag=f"xT{k}", name=f"xTs{k}")
                     for k in range(KC)]
            wv_sb = [wvp.tile([P, V_COLS], BF16, tag=f"wv{k}", name=f"wvs{k}")
                     for k in range(KC)]
            def emit_wr_dma(t):
                wt = []
                for k in range(KC):
                    w = wrot.tile([P, P], BF16, tag=f"wr{k}", name=f"wr{t}_{k}")
                    nc.gpsimd.dma_start(
                        w[:], wqk[k * P:(k + 1) * P, t * P:(t + 1) * P])
                    wt.append(w)
                return wt

            # first matmuls need the pair-0 weight slices: issue them ahead
            # of the x^T bulk on the gpsimd queue
            wt0 = emit_wr_dma(0)
            wt4 = emit_wr_dma(4)
            # sq-block-sliced so the first qk column groups can start after
            # ~1MB of x^T, striped across both DMA queues (two engines)
            # to roughly double the effective load rate
            for jsl in range(NJB):
                for k in range(KC):
                    q = nc.sync if k % 2 == 0 else nc.gpsimd
                    q.dma_start(
                        xT_sb[k][:, jsl * 512:(jsl + 1) * 512],
                        xT[k * P:(k + 1) * P, jsl * 512:(jsl + 1) * 512])
                if jsl == 0:
                    for k in range(KC):
                        q = nc.sync if k % 2 == 0 else nc.gpsimd
                        q.dma_start(wv_sb[k][:], wv[k * P:(k + 1) * P, :])

            def emit_qk_col(t, wt, fine=False):
                """One column tile of q^T (t<4) or k^T (t>=4): 4 psum groups.
                fine=True returns one thunk per matmul (32 thunks) so a
                group can be dribbled into B's per-chunk PE slack."""
                if t < 4:
                    dst = qT[t] = qkT_pool.tile([P, SEQ], F32R, tag="qTa",
                                                name=f"qT{t}")
                else:
                    dst = kT[t - 4] = qkT_pool.tile([P, SEQ], F32R, tag="kTa",
                                                    name=f"kT{t-4}")

                state = {}

                def one_mm(j, k):
                    def go():
                        if k == 0:
                            state[j] = psA.tile([P, 512], F32, tag="psA_t",
                                                name=f"psqk{t}_{j}")
                        ps = state[j]
                        nc.tensor.matmul(
                            ps[:], wt[k][:],
                            xT_sb[k][:, j * 512:(j + 1) * 512],
                            start=(k == 0), stop=(k == KC - 1))
                        if k == KC - 1:
                            nc.vector.tensor_scalar_add(
                                dst[:, j * 512:(j + 1) * 512], ps[:],
                                bqk_sb[:, t:t + 1])
                    return go

                def one_group(j):
                    def go():
                        for k in range(KC):
                            one_mm(j, k)()
                    return go

                if fine:
                    return [one_mm(j, k) for j in range(NJB) for k in range(KC)]
                return [one_group(j) for j in range(NJB)]

            def emit_v_group(s):
                ps = psA.tile([P, V_COLS], F32, tag="psA_t", name=f"psv{s}")
                for k in range(KC):
                    nc.tensor.matmul(
                        ps[:],
                        xT_sb[k][:, s * P:(s + 1) * P],
                        wv_sb[k][:],
                        start=(k == 0), stop=(k == KC - 1))
                v3 = vsb[s][:].rearrange("p (h c) -> p h c", c=VA)
                ps3 = ps[:].rearrange("p (h c) -> p h c", c=D_HEAD)
                bv3 = bv_sb[:].rearrange("p (h c) -> p h c", c=D_HEAD)
                nc.vector.tensor_add(v3[:, :, 0:D_HEAD], ps3, bv3)
                nc.vector.tensor_scalar(
                    v3[:, :, D_HEAD], bv_sb[:, 0:N_HEADS_CORE], 0.0, 1.0,
                    mybir.AluOpType.mult, mybir.AluOpType.add)

            # head: q^T/k^T for pair 0 and v, emitted j-major and
            # interleaved per xT slice so the PE starts after ~1MB of
            # x^T DMA instead of all 4MB
            wt01 = {}
            wt01[1] = emit_wr_dma(1)   # prefetch pair0's first filler weights
            q_groups = emit_qk_col(0, wt0)
            k_groups = emit_qk_col(4, wt4)
            for j in range(NJB):
                q_groups[j]()
                k_groups[j]()
                for s in range(4 * j, 4 * j + 4):
                    emit_v_group(s)

            # B pairs 0-2, with pair p+1's q^T/k^T production dribbled
            # into the chunk stream one matmul at a time; weight slices
            # prefetched one pair ahead
            for pair in range(3):
                t_lo, t_hi = pair + 1, pair + 5
                wt_lo = wt01.pop(t_lo, None) or emit_wr_dma(t_lo)
                fillers = list(emit_qk_col(t_lo, wt_lo, fine=True))
                wt_hi = emit_wr_dma(t_hi)
                if pair < 2:
                    wt01[pair + 2] = emit_wr_dma(pair + 2)
                fillers.extend(emit_qk_col(t_hi, wt_hi, fine=True))
                emit_B_pair(pair, fillers, psA)

        # ---- pair 3 + projection (xT/wv freed; wo loads into that space)
        with tc.tile_pool(name="wop", bufs=1) as wop, \
             tc.tile_pool(name="osb", bufs=2) as osb_pool, \
             tc.tile_pool(name="psC", bufs=2, space="PSUM") as psC:
            wo_sb = [wop.tile([P, EMB], F32R, tag=f"wo{t}", name=f"wo{t}")
                     for t in range(4)]
            for t in range(4):
                nc.sync.dma_start(wo_sb[t][:], wo[t * P:(t + 1) * P, :])

            cstate = {}

            def one_c_mm(s, y, t):
                def go():
                    if t == 0:
                        cstate[(s, y)] = psC.tile([P, 512], F32, tag="psC_t",
                                                  name=f"psc{s}_{y}")
                    ps = cstate[(s, y)]
                    nc.tensor.matmul(
                        ps[:],
                        outT[t][:, s * P:(s + 1) * P],
                        wo_sb[t][:, y * 512:(y + 1) * 512],
                        start=(t == 0), stop=(t == 3))
                    if t == 3:
                        ot = osb_pool.tile([P, 512], BF16, tag="osb",
                                           name=f"osb{s}_{y}")
                        nc.vector.tensor_copy(ot[:], ps[:])
                        q = nc.sync if (s + y) % 2 == 0 else nc.gpsimd
                        q.dma_start(
                            out[s * P:(s + 1) * P, y * 512:(y + 1) * 512],
                            ot[:])
                return go

            def emit_C_slice(s):
                return [one_c_mm(s, y, t)
                        for y in range(EMB // 512)
                        for t in range(4)]

            emit_B_pair(3, [], psC, after_jb=emit_C_slice, dynamic=True,
                        flush=True)

    nc.compile()
    return nc


def get_nc():
    global _CACHED
    if _CACHED is None:
        _CACHED = _build()
    return _CACHED


def make_in_maps(x, W_qkv, b_qkv, W_out, b_out):
    x = np.asarray(x, dtype=np.float32)
    W_qkv = np.asarray(W_qkv, dtype=np.float32)
    b_qkv = np.asarray(b_qkv, dtype=np.float32)
    W_out = np.asarray(W_out, dtype=np.float32)
    b_out = np.asarray(b_out, dtype=np.float32)

    in_maps = []
    for core in range(N_CORES):
        b, g = divmod(core, 2)
        c0 = g * 512
        wq = W_qkv[:, c0:c0 + 512] * NORM
        wk = W_qkv[:, EMB + c0:EMB + c0 + 512]
        wv_ = W_qkv[:, 2 * EMB + c0:2 * EMB + c0 + 512]
        bq = b_qkv[c0:c0 + 512] * NORM
        bk = b_qkv[EMB + c0:EMB + c0 + 512]
        bv_ = b_qkv[2 * EMB + c0:2 * EMB + c0 + 512]
        in_maps.append({
            "xT": np.ascontiguousarray(x[b].T).astype(BF16_NP),
            "wqk": np.ascontiguousarray(
                np.concatenate([wq, wk], axis=1)).astype(BF16_NP),
            "wv": np.ascontiguousarray(wv_).astype(BF16_NP),
            "wo": np.ascontiguousarray(W_out[c0:c0 + 512, :]),
            "bqk": np.ascontiguousarray(
                np.concatenate([bq, bk]).reshape(QK_COLS // P, P).T),
            "bv": bv_.reshape(1, V_COLS),
        })
    return in_maps


def kernel(x, W_qkv, b_qkv, W_out, b_out):
    nc = get_nc()
    b_out = np.asarray(b_out, dtype=np.float32)
    in_maps = make_in_maps(x, W_qkv, b_qkv, W_out, b_out)
    res = bass_utils.run_bass_kernel_spmd(nc, in_maps, core_ids=list(range(N_CORES)))
    outp = np.empty((4, SEQ, EMB), dtype=np.float32)
    for b in range(4):
        outp[b] = (res.results[2 * b]["out"].astype(np.float32)
                   + res.results[2 * b + 1]["out"].astype(np.float32) + b_out)
    return outp



# revision 32
# speedup vs baseline: 1.0257x; 1.0257x over previous
"""Multi-head attention (16 heads, E=1024, seq=2048, batch=4) on 8 NeuronCores.

Sharding: core = 2*b + g  (b = batch 0..3, g = head-group 0..1, 8 heads each).
Each core computes its batch's QKV for its 8 heads, attention, and a partial
output projection (rows of W_out for its heads); host sums the two partials
per batch and adds b_out.

On-chip layout avoids all transposes:
  - host supplies x^T [1024, 2048] per core
  - q^T,k^T computed as (W^T x^T)  -> [qk_col, seq]   (lhsT = W chunk)
  - v computed naturally as x @ W_v -> [seq, v_col]   (lhsT = x^T chunk)
  - scores^T[sk, sq] = (k^T chunk)^T.T @ q^T  (lhsT = k^T slice, rhs = q^T);
    head pairs share one PSUM tile ([A sq512 | B sq512]) with the two
    64-contraction matmuls row-packed via tile_position, so one Exp
    activation covers both heads
  - softmax denominator via an appended ones-column in the PV lhsT
  - PV: out^T[d(+1), sq] = [v | 1]^T @ attn^T, accumulated over sk chunks
  - normalize: denominator row broadcast across partitions with a K=1
    matmul, then reciprocal+multiply on DVE (deferred into the next
    sq-block's chunk stream so the PE never waits on the chain)
  - proj: y[sq, :] from lhsT = out^T tiles, rhs = W_out rows for this group

Scheduling: phase B (attention) is ACT-bound on the Exp stream, so the
QKV projections for head pair p+1 and the final projection are dribbled
into pair p's / pair 3's chunk stream one matmul at a time to fill the
PE slack. All matmuls run in float32r (TF32-like, full rate at N=512)
with fp32 PSUM accumulate.
"""

import sys

sys.path.insert(0, "/opt/trn_rl_repo")

import ml_dtypes
import numpy as np

BF16_NP = ml_dtypes.bfloat16

import concourse.bacc as bacc
import concourse.mybir as mybir
import concourse.tile as tile
from concourse import bass_utils

P = 128
SEQ = 2048
EMB = 1024
N_HEADS_CORE = 8
D_HEAD = 64
QK_COLS = 1024          # q(512) + k(512) for this core's heads
V_COLS = 512
VA = D_HEAD + 1         # v columns per head incl. ones column
N_CORES = 8
NORM = 0.125            # 1/sqrt(64), folded into W_q/b_q on host

F32 = mybir.dt.float32
F32R = mybir.dt.float32r
BF16 = mybir.dt.bfloat16
AF = mybir.ActivationFunctionType

_CACHED = None


def _build():
    nc = bacc.Bacc("TRN2", target_bir_lowering=False, debug=False,
                   enable_asserts=True, num_devices=N_CORES)

    xT = nc.dram_tensor("xT", [EMB, SEQ], BF16, kind="ExternalInput").ap()
    wqk = nc.dram_tensor("wqk", [EMB, QK_COLS], BF16, kind="ExternalInput").ap()
    wv = nc.dram_tensor("wv", [EMB, V_COLS], BF16, kind="ExternalInput").ap()
    wo = nc.dram_tensor("wo", [V_COLS, EMB], F32R, kind="ExternalInput").ap()
    bqk = nc.dram_tensor("bqk", [P, QK_COLS // P], F32, kind="ExternalInput").ap()
    bv = nc.dram_tensor("bv", [1, V_COLS], F32, kind="ExternalInput").ap()
    out = nc.dram_tensor("out", [SEQ, EMB], BF16, kind="ExternalOutput").ap()
    # DRAM bounce buffer for the softmax denominator rows: SBUF sources
    # cannot do stride-0 partition broadcast, DRAM sources can
    dnb = nc.dram_tensor("dnb", [2, 512], F32, kind="Internal").ap()

    KC = EMB // P          # 8 contraction chunks
    NSC = SEQ // P         # 16 seq chunks of 128
    NJB = SEQ // 512       # 4 sq blocks of 512

    with tile.TileContext(nc) as tc:
      with tc.tile_pool(name="persist", bufs=1) as persist, \
           tc.tile_pool(name="qkT", bufs=2) as qkT_pool, \
           tc.tile_pool(name="oTp", bufs=1) as oT_pool, \
           tc.tile_pool(name="attn", bufs=2) as attn_pool, \
           tc.tile_pool(name="nrm", bufs=2) as nrm_pool, \
           tc.tile_pool(name="ps_s", bufs=2, space="PSUM") as ps_s_pool, \
           tc.tile_pool(name="ps_o0", bufs=1, space="PSUM") as ps_o0_pool, \
           tc.tile_pool(name="ps_o1", bufs=1, space="PSUM") as ps_o1_pool:
        ps_o_pools = [ps_o0_pool, ps_o1_pool]
        vsb = [persist.tile([P, N_HEADS_CORE * VA], F32R, tag=f"v{s}", name=f"v{s}")
               for s in range(NSC)]
        bqk_sb = persist.tile([P, QK_COLS // P], F32, tag="bqk")
        bv_sb = persist.tile([P, V_COLS], F32, tag="bv")
        nc.sync.dma_start(bqk_sb[:], bqk)
        nc.sync.dma_start(bv_sb[:], bv[0:1, :].broadcast_to([P, V_COLS]))

        qT = {}
        kT = {}
        outT = [oT_pool.tile([P, SEQ], F32R, tag=f"oT{t}", name=f"oT{t}")
                for t in range(4)]

        from collections import deque
        pending = deque()

        def emit_B_pair(t, fillers, scratch_pool, after_jb=None, dynamic=False,
                        flush=False):
            """Head pair (2t, 2t+1): rows 0-63 / 64-127 of qT[t]/kT[t].
            Per chunk one ps_s [128,1024] = [A sq512 | B sq512]; scores
            row-packed, one exp for both heads, PV splits to per-head
            accumulators. `fillers` are thunks sprinkled into the chunk
            stream to fill PE slack under the ACT-bound exp pipeline."""
            kTh = kT[t]
            qTh = qT[t]
            it = 0
            fi = 0
            nfill = len(fillers)
            for j in range(NJB):
                sq0 = j * 512
                ps_os = [ps_o_pools[hh].tile([VA, 512], F32, tag=f"ps_o{hh}",
                                             name=f"ps_o{t}_{j}_{hh}")
                         for hh in range(2)]

                def scores(c):
                    ps_s = ps_s_pool.tile([P, 2 * 512], F32, tag="ps_s",
                                          name=f"ps_s{t}_{j}_{c}")
                    for hh in range(2):
                        pr = hh * D_HEAD
                        nc.tensor.matmul(
                            ps_s[:, hh * 512:(hh + 1) * 512],
                            kTh[pr:pr + D_HEAD, c * P:(c + 1) * P],
                            qTh[pr:pr + D_HEAD, sq0:sq0 + 512],
                            start=True, stop=True, tile_position=(pr, 0))
                    return ps_s

                ps_s = scores(0)
                for c in range(NSC):
                    at = attn_pool.tile([P, 2 * 512], F32R, tag="attnT",
                                        name=f"at{t}_{j}_{c}")
                    nc.scalar.activation(at[:], ps_s[:], AF.Exp)
                    if c + 1 < NSC:
                        ps_s = scores(c + 1)
                    va3 = vsb[c][:].rearrange("p (h c) -> p h c", c=VA)
                    for hh in range(2):
                        nc.tensor.matmul(
                            ps_os[hh][:],
                            va3[:, 2 * t + hh, :],
                            at[:, hh * 512:(hh + 1) * 512],
                            start=(c == 0), stop=(c == NSC - 1))
                    it += 1
                    if c >= 2:
                        for _ in range(4):
                            if pending:
                                pending.popleft()()
                    if dynamic:
                        budget = 3
                        while fi < len(fillers) and budget > 0:
                            fillers[fi]()
                            fi += 1
                            budget -= 1
                    else:
                        while nfill and fi < (nfill * it) // 64 and fi < nfill:
                            fillers[fi]()
                            fi += 1

                # stage 1 (DVE): evacuate ps_o; broadcast the denominator
                # row across 64 partitions with a stride-0 SBUF->SBUF DMA
                # (keeps the normalize chain off PSUM and off the PE)
                outUs = []
                rcasts = []
                for hh in range(2):
                    outU = nrm_pool.tile([VA, 512], F32, tag=f"outU{hh}",
                                         name=f"outU{t}_{j}_{hh}", bufs=1)
                    nc.vector.tensor_copy(outU[:], ps_os[hh][:])
                    rbb = nrm_pool.tile([D_HEAD, 512], F32, tag=f"rbb{hh}",
                                        name=f"rbb{t}_{j}_{hh}", bufs=1)
                    # gpsimd DMA queue: the sync queue carries the output
                    # tiles in pair 3 and would delay this bounce by ~4us
                    nc.gpsimd.dma_start(dnb[hh:hh + 1, :], outU[D_HEAD:VA, :])
                    nc.gpsimd.dma_start(
                        rbb[:], dnb[hh:hh + 1, :].broadcast_to([D_HEAD, 512]))
                    outUs.append(outU)
                    rcasts.append(rbb)

                # stage 2, deferred into the next j-block's chunk stream as
                # small steps: reciprocal+multiply in [64,128] column slices
                # so no single DVE op can block the FIFO ahead of the PSUM
                # evacuations that gate the PE.
                def make_fin_steps(jj, sq00, oUs, rcs):
                    def recmul(hh, p):
                        def go():
                            cs = slice(p * 128, (p + 1) * 128)
                            rb = nrm_pool.tile([D_HEAD, 128], F32,
                                               tag=f"rb{hh}_{p}",
                                               name=f"rb{t}_{jj}_{hh}_{p}",
                                               bufs=1)
                            nc.vector.reciprocal(rb[:], rcs[hh][:, cs])
                            nc.vector.tensor_mul(
                                outT[t][hh * D_HEAD:(hh + 1) * D_HEAD,
                                        sq00 + p * 128:sq00 + (p + 1) * 128],
                                oUs[hh][0:D_HEAD, cs], rb[:])
                        return go

                    steps = []
                    for p in range(4):
                        for hh in range(2):
                            steps.append(recmul(hh, p))
                        if after_jb is not None:
                            # proj thunks for this column slice right after
                            # the muls that produce their outT inputs
                            steps.extend(after_jb(4 * jj + p))
                    return steps

                pending.extend(make_fin_steps(j, sq0, outUs, rcasts))
            # drain: alternate steps and fillers so the tail's proj matmuls
            # (extended by the last step) overlap the DVE normalize chain
            while (flush and pending) or fi < len(fillers):
                if flush and pending:
                    pending.popleft()()
                for _ in range(3):
                    if fi < len(fillers):
                        fillers[fi]()
                        fi += 1

        # ---- phase A scaffolding (xT, wv, rotating weight slices) ----
        with tc.tile_pool(name="xTp", bufs=1) as xTp, \
             tc.tile_pool(name="wvp", bufs=1) as wvp, \
             tc.tile_pool(name="wrot", bufs=2) as wrot, \
             tc.tile_pool(name="psA", bufs=2, space="PSUM") as psA:
            xT_sb = [xTp.tile([P, SEQ], BF16, tag=f"xT{k}", name=f"xTs{k}")
                     for k in range(KC)]
            wv_sb = [wvp.tile([P, V_COLS], BF16, tag=f"wv{k}", name=f"wvs{k}")
                     for k in range(KC)]
            def emit_wr_dma(t):
                wt = []
                for k in range(KC):
                    w = wrot.tile([P, P], BF16, tag=f"wr{k}", name=f"wr{t}_{k}")
                    nc.gpsimd.dma_start(
                        w[:], wqk[k * P:(k + 1) * P, t * P:(t + 1) * P])
                    wt.append(w)
                return wt

            # first matmuls need the pair-0 weight slices: issue them ahead
            # of the x^T bulk on the gpsimd queue
            wt0 = emit_wr_dma(0)
            wt4 = emit_wr_dma(4)
            # x^T in two half-tile sweeps of [128,1024] (256KB descriptors
            # are transfer-bound, 128KB ones are issue-bound at ~600ns per
            # dma_start); wv between the halves so v groups can start
            for h in range(2):
                for k in range(KC):
                    nc.sync.dma_start(
                        xT_sb[k][:, h * 1024:(h + 1) * 1024],
                        xT[k * P:(k + 1) * P, h * 1024:(h + 1) * 1024])
                if h == 0:
                    for k in range(KC):
                        nc.sync.dma_start(wv_sb[k][:],
                                          wv[k * P:(k + 1) * P, :])

            def emit_qk_col(t, wt, fine=False):
                """One column tile of q^T (t<4) or k^T (t>=4): 4 psum groups.
                fine=True returns one thunk per matmul (32 thunks) so a
                group can be dribbled into B's per-chunk PE slack."""
                if t < 4:
                    dst = qT[t] = qkT_pool.tile([P, SEQ], F32R, tag="qTa",
                                                name=f"qT{t}")
                else:
                    dst = kT[t - 4] = qkT_pool.tile([P, SEQ], F32R, tag="kTa",
                                                    name=f"kT{t-4}")

                state = {}

                def one_mm(j, k):
                    def go():
                        if k == 0:
                            state[j] = psA.tile([P, 512], F32, tag="psA_t",
                                                name=f"psqk{t}_{j}")
                        ps = state[j]
                        nc.tensor.matmul(
                            ps[:], wt[k][:],
                            xT_sb[k][:, j * 512:(j + 1) * 512],
                            start=(k == 0), stop=(k == KC - 1))
                        if k == KC - 1:
                            nc.vector.tensor_scalar_add(
                                dst[:, j * 512:(j + 1) * 512], ps[:],
                                bqk_sb[:, t:t + 1])
                    return go

                def one_group(j):
                    def go():
                        for k in range(KC):
                            one_mm(j, k)()
                    return go

                if fine:
                    return [one_mm(j, k) for j in range(NJB) for k in range(KC)]
                return [one_group(j) for j in range(NJB)]

            def emit_v_group(s):
                ps = psA.tile([P, V_COLS], F32, tag="psA_t", name=f"psv{s}")
                for k in range(KC):
                    nc.tensor.matmul(
                        ps[:],
                        xT_sb[k][:, s * P:(s + 1) * P],
                        wv_sb[k][:],
                        start=(k == 0), stop=(k == KC - 1))
                v3 = vsb[s][:].rearrange("p (h c) -> p h c", c=VA)
                ps3 = ps[:].rearrange("p (h c) -> p h c", c=D_HEAD)
                bv3 = bv_sb[:].rearrange("p (h c) -> p h c", c=D_HEAD)
                nc.vector.tensor_add(v3[:, :, 0:D_HEAD], ps3, bv3)
                nc.vector.tensor_scalar(
                    v3[:, :, D_HEAD], bv_sb[:, 0:N_HEADS_CORE], 0.0, 1.0,
                    mybir.AluOpType.mult, mybir.AluOpType.add)

            # head: q^T/k^T for pair 0 and v, emitted j-major and
            # interleaved per xT slice so the PE starts after ~1MB of
            # x^T DMA instead of all 4MB
            wt01 = {}
            wt01[1] = emit_wr_dma(1)   # prefetch pair0's first filler weights
            q_groups = emit_qk_col(0, wt0)
            k_groups = emit_qk_col(4, wt4)
            for j in range(NJB):
                q_groups[j]()
                k_groups[j]()
                for s in range(4 * j, 4 * j + 4):
                    emit_v_group(s)

            # B pairs 0-2, with pair p+1's q^T/k^T production dribbled
            # into the chunk stream one matmul at a time; weight slices
            # prefetched one pair ahead
            for pair in range(3):
                t_lo, t_hi = pair + 1, pair + 5
                wt_lo = wt01.pop(t_lo, None) or emit_wr_dma(t_lo)
                fillers = list(emit_qk_col(t_lo, wt_lo, fine=True))
                wt_hi = emit_wr_dma(t_hi)
                if pair < 2:
                    wt01[pair + 2] = emit_wr_dma(pair + 2)
                fillers.extend(emit_qk_col(t_hi, wt_hi, fine=True))
                emit_B_pair(pair, fillers, psA)

        # ---- pair 3 + projection (xT/wv freed; wo loads into that space)
        with tc.tile_pool(name="wop", bufs=1) as wop, \
             tc.tile_pool(name="osb", bufs=2) as osb_pool, \
             tc.tile_pool(name="psC", bufs=2, space="PSUM") as psC:
            wo_sb = [wop.tile([P, EMB], F32R, tag=f"wo{t}", name=f"wo{t}")
                     for t in range(4)]
            for t in range(4):
                nc.sync.dma_start(wo_sb[t][:], wo[t * P:(t + 1) * P, :])

            cstate = {}

            def one_c_mm(s, y, t):
                def go():
                    if t == 0:
                        cstate[(s, y)] = psC.tile([P, 512], F32, tag="psC_t",
                                                  name=f"psc{s}_{y}")
                    ps = cstate[(s, y)]
                    nc.tensor.matmul(
                        ps[:],
                        outT[t][:, s * P:(s + 1) * P],
                        wo_sb[t][:, y * 512:(y + 1) * 512],
                        start=(t == 0), stop=(t == 3))
                    if t == 3:
                        if y == 0:
                            cstate[s] = osb_pool.tile(
                                [P, EMB], BF16, tag="osb", name=f"osb{s}")
                        ot = cstate[s]
                        nc.vector.tensor_copy(
                            ot[:, y * 512:(y + 1) * 512], ps[:])
                        if y == 1:
                            # one 256KB transfer-bound DMA per row block
                            nc.sync.dma_start(
                                out[s * P:(s + 1) * P, :], ot[:])
                return go

            def emit_C_slice(s):
                return [one_c_mm(s, y, t)
                        for y in range(EMB // 512)
                        for t in range(4)]

            emit_B_pair(3, [], psC, after_jb=emit_C_slice, dynamic=True,
                        flush=True)

    nc.compile()
    return nc


def get_nc():
    global _CACHED
    if _CACHED is None:
        _CACHED = _build()
    return _CACHED


def make_in_maps(x, W_qkv, b_qkv, W_out, b_out):
    x = np.asarray(x, dtype=np.float32)
    W_qkv = np.asarray(W_qkv, dtype=np.float32)
    b_qkv = np.asarray(b_qkv, dtype=np.float32)
    W_out = np.asarray(W_out, dtype=np.float32)
    b_out = np.asarray(b_out, dtype=np.float32)

    in_maps = []
    for core in range(N_CORES):
        b, g = divmod(core, 2)
        c0 = g * 512
        wq = W_qkv[:, c0:c0 + 512] * NORM
        wk = W_qkv[:, EMB + c0:EMB + c0 + 512]
        wv_ = W_qkv[:, 2 * EMB + c0:2 * EMB + c0 + 512]
        bq = b_qkv[c0:c0 + 512] * NORM
        bk = b_qkv[EMB + c0:EMB + c0 + 512]
        bv_ = b_qkv[2 * EMB + c0:2 * EMB + c0 + 512]
        in_maps.append({
            "xT": np.ascontiguousarray(x[b].T).astype(BF16_NP),
            "wqk": np.ascontiguousarray(
                np.concatenate([wq, wk], axis=1)).astype(BF16_NP),
            "wv": np.ascontiguousarray(wv_).astype(BF16_NP),
            "wo": np.ascontiguousarray(W_out[c0:c0 + 512, :]),
            "bqk": np.ascontiguousarray(
                np.concatenate([bq, bk]).reshape(QK_COLS // P, P).T),
            "bv": bv_.reshape(1, V_COLS),
        })
    return in_maps


def kernel(x, W_qkv, b_qkv, W_out, b_out):
    nc = get_nc()
    b_out = np.asarray(b_out, dtype=np.float32)
    in_maps = make_in_maps(x, W_qkv, b_qkv, W_out, b_out)
    res = bass_utils.run_bass_kernel_spmd(nc, in_maps, core_ids=list(range(N_CORES)))
    outp = np.empty((4, SEQ, EMB), dtype=np.float32)
    for b in range(4):
        outp[b] = (res.results[2 * b]["out"].astype(np.float32)
                   + res.results[2 * b + 1]["out"].astype(np.float32) + b_out)
    return outp



# revision 36
# speedup vs baseline: 1.0334x; 1.0074x over previous
"""Multi-head attention (16 heads, E=1024, seq=2048, batch=4) on 8 NeuronCores.

Sharding: core = 2*b + g  (b = batch 0..3, g = head-group 0..1, 8 heads each).
Each core computes its batch's QKV for its 8 heads, attention, and a partial
output projection (rows of W_out for its heads); host sums the two partials
per batch and adds b_out.

On-chip layout avoids all transposes:
  - host supplies x^T [1024, 2048] per core
  - q^T,k^T computed as (W^T x^T)  -> [qk_col, seq]   (lhsT = W chunk)
  - v computed naturally as x @ W_v -> [seq, v_col]   (lhsT = x^T chunk)
  - scores^T[sk, sq] = (k^T chunk)^T.T @ q^T  (lhsT = k^T slice, rhs = q^T);
    head pairs share one PSUM tile ([A sq512 | B sq512]) with the two
    64-contraction matmuls row-packed via tile_position, so one Exp
    activation covers both heads
  - softmax denominator via an appended ones-column in the PV lhsT
  - PV: out^T[d(+1), sq] = [v | 1]^T @ attn^T, accumulated over sk chunks
  - normalize: denominator row broadcast across partitions with a K=1
    matmul, then reciprocal+multiply on DVE (deferred into the next
    sq-block's chunk stream so the PE never waits on the chain)
  - proj: y[sq, :] from lhsT = out^T tiles, rhs = W_out rows for this group

Scheduling: phase B (attention) is ACT-bound on the Exp stream, so the
QKV projections for head pair p+1 and the final projection are dribbled
into pair p's / pair 3's chunk stream one matmul at a time to fill the
PE slack. All matmuls run in float32r (TF32-like, full rate at N=512)
with fp32 PSUM accumulate.
"""

import sys

sys.path.insert(0, "/opt/trn_rl_repo")

import ml_dtypes
import numpy as np

BF16_NP = ml_dtypes.bfloat16

import concourse.bacc as bacc
import concourse.mybir as mybir
import concourse.tile as tile
from concourse import bass_utils

P = 128
SEQ = 2048
EMB = 1024
N_HEADS_CORE = 8
D_HEAD = 64
QK_COLS = 1024          # q(512) + k(512) for this core's heads
V_COLS = 512
VA = D_HEAD + 1         # v columns per head incl. ones column
N_CORES = 8
NORM = 0.125            # 1/sqrt(64), folded into W_q/b_q on host

F32 = mybir.dt.float32
F32R = mybir.dt.float32r
BF16 = mybir.dt.bfloat16
AF = mybir.ActivationFunctionType

_CACHED = None


def _build():
    nc = bacc.Bacc("TRN2", target_bir_lowering=False, debug=False,
                   enable_asserts=True, num_devices=N_CORES)

    xT = nc.dram_tensor("xT", [EMB, SEQ], BF16, kind="ExternalInput").ap()
    wqk = nc.dram_tensor("wqk", [EMB, QK_COLS], BF16, kind="ExternalInput").ap()
    wv = nc.dram_tensor("wv", [EMB, V_COLS], BF16, kind="ExternalInput").ap()
    wo = nc.dram_tensor("wo", [V_COLS, EMB], F32R, kind="ExternalInput").ap()
    bqk = nc.dram_tensor("bqk", [P, QK_COLS // P], F32, kind="ExternalInput").ap()
    bv = nc.dram_tensor("bv", [1, V_COLS], F32, kind="ExternalInput").ap()
    out = nc.dram_tensor("out", [SEQ, EMB], BF16, kind="ExternalOutput").ap()
    # DRAM bounce buffer for the softmax denominator rows: SBUF sources
    # cannot do stride-0 partition broadcast, DRAM sources can
    dnb = nc.dram_tensor("dnb", [2, 512], F32, kind="Internal").ap()

    KC = EMB // P          # 8 contraction chunks
    NSC = SEQ // P         # 16 seq chunks of 128
    NJB = SEQ // 512       # 4 sq blocks of 512

    with tile.TileContext(nc) as tc:
      with tc.tile_pool(name="persist", bufs=1) as persist, \
           tc.tile_pool(name="qkT", bufs=2) as qkT_pool, \
           tc.tile_pool(name="oTp", bufs=1) as oT_pool, \
           tc.tile_pool(name="attn", bufs=2) as attn_pool, \
           tc.tile_pool(name="nrm", bufs=2) as nrm_pool, \
           tc.tile_pool(name="ps_s", bufs=2, space="PSUM") as ps_s_pool, \
           tc.tile_pool(name="ps_o0", bufs=1, space="PSUM") as ps_o0_pool, \
           tc.tile_pool(name="ps_o1", bufs=1, space="PSUM") as ps_o1_pool:
        ps_o_pools = [ps_o0_pool, ps_o1_pool]
        vsb = [persist.tile([P, N_HEADS_CORE * VA], F32R, tag=f"v{s}", name=f"v{s}")
               for s in range(NSC)]
        bqk_sb = persist.tile([P, QK_COLS // P], F32, tag="bqk")
        bv_sb = persist.tile([P, V_COLS], F32, tag="bv")
        nc.sync.dma_start(bqk_sb[:], bqk)
        nc.sync.dma_start(bv_sb[:], bv[0:1, :].broadcast_to([P, V_COLS]))

        ones_sb = persist.tile([P, D_HEAD], F32R, tag="ones")
        nc.vector.tensor_scalar(ones_sb[:], bv_sb[:, 0:D_HEAD], 0.0, 1.0,
                                mybir.AluOpType.mult, mybir.AluOpType.add)

        qT = {}
        kT = {}
        outT = [oT_pool.tile([P, SEQ], F32R, tag=f"oT{t}", name=f"oT{t}")
                for t in range(4)]

        from collections import deque
        pending = deque()

        def emit_B_pair(t, fillers, scratch_pool, after_jb=None, dynamic=False,
                        flush=False):
            """Head pair (2t, 2t+1): rows 0-63 / 64-127 of qT[t]/kT[t].
            Per chunk one ps_s [128,1024] = [A sq512 | B sq512]; scores
            row-packed, one exp for both heads, PV splits to per-head
            accumulators. `fillers` are thunks sprinkled into the chunk
            stream to fill PE slack under the ACT-bound exp pipeline."""
            kTh = kT[t]
            qTh = qT[t]
            it = 0
            fi = 0
            nfill = len(fillers)
            for j in range(NJB):
                sq0 = j * 512
                ps_os = [ps_o_pools[hh].tile([VA, 512], F32, tag=f"ps_o{hh}",
                                             name=f"ps_o{t}_{j}_{hh}")
                         for hh in range(2)]

                def scores(c):
                    ps_s = ps_s_pool.tile([P, 2 * 512], F32, tag="ps_s",
                                          name=f"ps_s{t}_{j}_{c}")
                    for hh in range(2):
                        pr = hh * D_HEAD
                        nc.tensor.matmul(
                            ps_s[:, hh * 512:(hh + 1) * 512],
                            kTh[pr:pr + D_HEAD, c * P:(c + 1) * P],
                            qTh[pr:pr + D_HEAD, sq0:sq0 + 512],
                            start=True, stop=True, tile_position=(pr, 0))
                    return ps_s

                ps_s = scores(0)
                for c in range(NSC):
                    at = attn_pool.tile([P, 2 * 512], F32R, tag="attnT",
                                        name=f"at{t}_{j}_{c}")
                    nc.scalar.activation(at[:], ps_s[:], AF.Exp)
                    if c + 1 < NSC:
                        ps_s = scores(c + 1)
                    va3 = vsb[c][:].rearrange("p (h c) -> p h c", c=VA)
                    for hh in range(2):
                        nc.tensor.matmul(
                            ps_os[hh][:],
                            va3[:, 2 * t + hh, :],
                            at[:, hh * 512:(hh + 1) * 512],
                            start=(c == 0), stop=(c == NSC - 1))
                    it += 1
                    if c >= 2:
                        for _ in range(4):
                            if pending:
                                pending.popleft()()
                    if dynamic:
                        budget = 3
                        while fi < len(fillers) and budget > 0:
                            fillers[fi]()
                            fi += 1
                            budget -= 1
                    else:
                        while nfill and fi < (nfill * it) // 64 and fi < nfill:
                            fillers[fi]()
                            fi += 1

                # stage 1 (DVE): evacuate ps_o; broadcast the denominator
                # row across 64 partitions with a stride-0 SBUF->SBUF DMA
                # (keeps the normalize chain off PSUM and off the PE)
                last_jb = flush and j == NJB - 1
                outUs = []
                rcasts = []
                for hh in range(2):
                    outU = nrm_pool.tile([VA, 512], F32, tag=f"outU{hh}",
                                         name=f"outU{t}_{j}_{hh}", bufs=1)
                    nc.vector.tensor_copy(outU[:], ps_os[hh][:])
                    if last_jb:
                        # tail: the DRAM bounce's ~7us of serial DMA legs
                        # would sit on the critical path; use the PE
                        # broadcast instead (psC ring is no longer paced)
                        rcst = nrm_pool.tile([VA, 512], F32R,
                                             tag=f"rcast{hh}",
                                             name=f"rcast{t}_{j}_{hh}",
                                             bufs=1)
                        with nc.allow_low_precision(reason="denom f32r"):
                            nc.vector.tensor_copy(rcst[D_HEAD:VA, :],
                                                  outU[D_HEAD:VA, :])
                        rcasts.append(rcst)
                    else:
                        rbb = nrm_pool.tile([D_HEAD, 512], F32,
                                            tag=f"rbb{hh}",
                                            name=f"rbb{t}_{j}_{hh}", bufs=1)
                        # gpsimd DMA queue: the sync queue carries the
                        # output tiles in pair 3 and would delay this
                        nc.gpsimd.dma_start(dnb[hh:hh + 1, :],
                                            outU[D_HEAD:VA, :])
                        nc.gpsimd.dma_start(
                            rbb[:],
                            dnb[hh:hh + 1, :].broadcast_to([D_HEAD, 512]))
                        rcasts.append(rbb)
                    outUs.append(outU)

                # stage 2, deferred into the next j-block's chunk stream as
                # small steps: reciprocal+multiply in [64,128] column slices
                # so no single DVE op can block the FIFO ahead of the PSUM
                # evacuations that gate the PE.
                def make_fin_steps(jj, sq00, oUs, rcs, pe_bcast):
                    psbs = {}

                    def bcast(hh):
                        def go():
                            psb = scratch_pool.tile(
                                [P, 512], F32, tag=scratch_pool.name + "_t",
                                name=f"psb{t}_{jj}_{hh}")
                            nc.tensor.matmul(psb[0:D_HEAD, :],
                                             ones_sb[D_HEAD:D_HEAD + 1, :],
                                             rcs[hh][D_HEAD:VA, :],
                                             start=True, stop=True,
                                             tile_position=(D_HEAD, 0))
                            psbs[hh] = psb
                        return go

                    def recmul(hh, p):
                        def go():
                            cs = slice(p * 128, (p + 1) * 128)
                            src = (psbs[hh][0:D_HEAD, cs] if pe_bcast
                                   else rcs[hh][:, cs])
                            rb = nrm_pool.tile([D_HEAD, 128], F32,
                                               tag=f"rb{hh}_{p}",
                                               name=f"rb{t}_{jj}_{hh}_{p}",
                                               bufs=1)
                            nc.vector.reciprocal(rb[:], src)
                            nc.vector.tensor_mul(
                                outT[t][hh * D_HEAD:(hh + 1) * D_HEAD,
                                        sq00 + p * 128:sq00 + (p + 1) * 128],
                                oUs[hh][0:D_HEAD, cs], rb[:])
                        return go

                    steps = []
                    if pe_bcast:
                        steps.extend([bcast(0), bcast(1)])
                    for p in range(4):
                        for hh in range(2):
                            steps.append(recmul(hh, p))
                        if after_jb is not None:
                            # proj thunks for this column slice right after
                            # the muls that produce their outT inputs
                            steps.extend(after_jb(4 * jj + p))
                    return steps

                pending.extend(make_fin_steps(j, sq0, outUs, rcasts,
                                              last_jb))
            # drain: alternate steps and fillers so the tail's proj matmuls
            # (extended by the last step) overlap the DVE normalize chain
            while (flush and pending) or fi < len(fillers):
                if flush and pending:
                    pending.popleft()()
                for _ in range(3):
                    if fi < len(fillers):
                        fillers[fi]()
                        fi += 1

        # ---- phase A scaffolding (xT, wv, rotating weight slices) ----
        with tc.tile_pool(name="xTp", bufs=1) as xTp, \
             tc.tile_pool(name="wvp", bufs=1) as wvp, \
             tc.tile_pool(name="wrot", bufs=2) as wrot, \
             tc.tile_pool(name="psA", bufs=2, space="PSUM") as psA:
            xT_sb = [xTp.tile([P, SEQ], BF16, tag=f"xT{k}", name=f"xTs{k}")
                     for k in range(KC)]
            wv_sb = [wvp.tile([P, V_COLS], BF16, tag=f"wv{k}", name=f"wvs{k}")
                     for k in range(KC)]
            def emit_wr_dma(t):
                wt = []
                for k in range(KC):
                    w = wrot.tile([P, P], BF16, tag=f"wr{k}", name=f"wr{t}_{k}")
                    nc.gpsimd.dma_start(
                        w[:], wqk[k * P:(k + 1) * P, t * P:(t + 1) * P])
                    wt.append(w)
                return wt

            # first matmuls need the pair-0 weight slices: issue them ahead
            # of the x^T bulk on the gpsimd queue
            wt0 = emit_wr_dma(0)
            wt4 = emit_wr_dma(4)
            # x^T in two half-tile sweeps of [128,1024] (256KB descriptors
            # are transfer-bound, 128KB ones are issue-bound at ~600ns per
            # dma_start); wv between the halves so v groups can start
            for h in range(2):
                for k in range(KC):
                    nc.sync.dma_start(
                        xT_sb[k][:, h * 1024:(h + 1) * 1024],
                        xT[k * P:(k + 1) * P, h * 1024:(h + 1) * 1024])
                if h == 0:
                    for k in range(KC):
                        nc.sync.dma_start(wv_sb[k][:],
                                          wv[k * P:(k + 1) * P, :])

            def emit_qk_col(t, wt, fine=False):
                """One column tile of q^T (t<4) or k^T (t>=4): 4 psum groups.
                fine=True returns one thunk per matmul (32 thunks) so a
                group can be dribbled into B's per-chunk PE slack."""
                if t < 4:
                    dst = qT[t] = qkT_pool.tile([P, SEQ], F32R, tag="qTa",
                                                name=f"qT{t}")
                else:
                    dst = kT[t - 4] = qkT_pool.tile([P, SEQ], F32R, tag="kTa",
                                                    name=f"kT{t-4}")

                state = {}

                def one_mm(j, k):
                    def go():
                        if k == 0:
                            state[j] = psA.tile([P, 512], F32, tag="psA_t",
                                                name=f"psqk{t}_{j}")
                        ps = state[j]
                        nc.tensor.matmul(
                            ps[:], wt[k][:],
                            xT_sb[k][:, j * 512:(j + 1) * 512],
                            start=(k == 0), stop=(k == KC - 1))
                        if k == KC - 1:
                            nc.vector.tensor_scalar_add(
                                dst[:, j * 512:(j + 1) * 512], ps[:],
                                bqk_sb[:, t:t + 1])
                    return go

                def one_group(j):
                    def go():
                        for k in range(KC):
                            one_mm(j, k)()
                    return go

                if fine:
                    return [one_mm(j, k) for j in range(NJB) for k in range(KC)]
                return [one_group(j) for j in range(NJB)]

            def emit_v_group(s):
                ps = psA.tile([P, V_COLS], F32, tag="psA_t", name=f"psv{s}")
                for k in range(KC):
                    nc.tensor.matmul(
                        ps[:],
                        xT_sb[k][:, s * P:(s + 1) * P],
                        wv_sb[k][:],
                        start=(k == 0), stop=(k == KC - 1))
                v3 = vsb[s][:].rearrange("p (h c) -> p h c", c=VA)
                ps3 = ps[:].rearrange("p (h c) -> p h c", c=D_HEAD)
                bv3 = bv_sb[:].rearrange("p (h c) -> p h c", c=D_HEAD)
                nc.vector.tensor_add(v3[:, :, 0:D_HEAD], ps3, bv3)
                nc.vector.tensor_scalar(
                    v3[:, :, D_HEAD], bv_sb[:, 0:N_HEADS_CORE], 0.0, 1.0,
                    mybir.AluOpType.mult, mybir.AluOpType.add)

            # head: q^T/k^T for pair 0 and v, emitted j-major and
            # interleaved per xT slice so the PE starts after ~1MB of
            # x^T DMA instead of all 4MB
            wt01 = {}
            wt01[1] = emit_wr_dma(1)   # prefetch pair0's first filler weights
            q_groups = emit_qk_col(0, wt0)
            k_groups = emit_qk_col(4, wt4)
            for j in range(NJB):
                q_groups[j]()
                k_groups[j]()
                for s in range(4 * j, 4 * j + 4):
                    emit_v_group(s)

            # B pairs 0-2, with pair p+1's q^T/k^T production dribbled
            # into the chunk stream one matmul at a time; weight slices
            # prefetched one pair ahead
            for pair in range(3):
                t_lo, t_hi = pair + 1, pair + 5
                wt_lo = wt01.pop(t_lo, None) or emit_wr_dma(t_lo)
                fillers = list(emit_qk_col(t_lo, wt_lo, fine=True))
                wt_hi = emit_wr_dma(t_hi)
                if pair < 2:
                    wt01[pair + 2] = emit_wr_dma(pair + 2)
                fillers.extend(emit_qk_col(t_hi, wt_hi, fine=True))
                emit_B_pair(pair, fillers, psA)

        # ---- pair 3 + projection (xT/wv freed; wo loads into that space)
        with tc.tile_pool(name="wop", bufs=1) as wop, \
             tc.tile_pool(name="osb", bufs=2) as osb_pool, \
             tc.tile_pool(name="psC", bufs=2, space="PSUM") as psC:
            wo_sb = [wop.tile([P, EMB], F32R, tag=f"wo{t}", name=f"wo{t}")
                     for t in range(4)]
            for t in range(4):
                nc.sync.dma_start(wo_sb[t][:], wo[t * P:(t + 1) * P, :])

            cstate = {}

            def one_c_mm(s, y, t):
                def go():
                    if t == 0:
                        cstate[(s, y)] = psC.tile([P, 512], F32, tag="psC_t",
                                                  name=f"psc{s}_{y}")
                    ps = cstate[(s, y)]
                    nc.tensor.matmul(
                        ps[:],
                        outT[t][:, s * P:(s + 1) * P],
                        wo_sb[t][:, y * 512:(y + 1) * 512],
                        start=(t == 0), stop=(t == 3))
                    if t == 3:
                        if y == 0:
                            cstate[s] = osb_pool.tile(
                                [P, EMB], BF16, tag="osb", name=f"osb{s}")
                        ot = cstate[s]
                        nc.vector.tensor_copy(
                            ot[:, y * 512:(y + 1) * 512], ps[:])
                        if y == 1:
                            # one 256KB transfer-bound DMA per row block
                            nc.sync.dma_start(
                                out[s * P:(s + 1) * P, :], ot[:])
                return go

            def emit_C_slice(s):
                return [one_c_mm(s, y, t)
                        for y in range(EMB // 512)
                        for t in range(4)]

            emit_B_pair(3, [], psC, after_jb=emit_C_slice, dynamic=True,
                        flush=True)

    nc.compile()
    return nc


def get_nc():
    global _CACHED
    if _CACHED is None:
        _CACHED = _build()
    return _CACHED


def make_in_maps(x, W_qkv, b_qkv, W_out, b_out):
    x = np.asarray(x, dtype=np.float32)
    W_qkv = np.asarray(W_qkv, dtype=np.float32)
    b_qkv = np.asarray(b_qkv, dtype=np.float32)
    W_out = np.asarray(W_out, dtype=np.float32)
    b_out = np.asarray(b_out, dtype=np.float32)

    in_maps = []
    for core in range(N_CORES):
        b, g = divmod(core, 2)
        c0 = g * 512
        wq = W_qkv[:, c0:c0 + 512] * NORM
        wk = W_qkv[:, EMB + c0:EMB + c0 + 512]
        wv_ = W_qkv[:, 2 * EMB + c0:2 * EMB + c0 + 512]
        bq = b_qkv[c0:c0 + 512] * NORM
        bk = b_qkv[EMB + c0:EMB + c0 + 512]
        bv_ = b_qkv[2 * EMB + c0:2 * EMB + c0 + 512]
        in_maps.append({
            "xT": np.ascontiguousarray(x[b].T).astype(BF16_NP),
            "wqk": np.ascontiguousarray(
                np.concatenate([wq, wk], axis=1)).astype(BF16_NP),
            "wv": np.ascontiguousarray(wv_).astype(BF16_NP),
            "wo": np.ascontiguousarray(W_out[c0:c0 + 512, :]),
            "bqk": np.ascontiguousarray(
                np.concatenate([bq, bk]).reshape(QK_COLS // P, P).T),
            "bv": bv_.reshape(1, V_COLS),
        })
    return in_maps


def kernel(x, W_qkv, b_qkv, W_out, b_out):
    nc = get_nc()
    b_out = np.asarray(b_out, dtype=np.float32)
    in_maps = make_in_maps(x, W_qkv, b_qkv, W_out, b_out)
    res = bass_utils.run_bass_kernel_spmd(nc, in_maps, core_ids=list(range(N_CORES)))
    outp = np.empty((4, SEQ, EMB), dtype=np.float32)
    for b in range(4):
        outp[b] = (res.results[2 * b]["out"].astype(np.float32)
                   + res.results[2 * b + 1]["out"].astype(np.float32) + b_out)
    return outp

